# revision 1
# baseline (speedup 1.0000x reference)
"""Trainium2 Bass kernel for nn_Cycle_Consistency_Loss (soft-DTW-style
cycle loss). Self-contained: host-side packing + SPMD Bass program on 8
NeuronCores + host reduction.

Math (per pair (a,b), both directions; x = seq[q], y = seq[k], lens = src_len//4):
  alpha = softmax_j(-|x_i-y_j|^2) over valid j -> snn = alpha @ y
  beta  = softmax_k(-|snn_i-x_k|^2) over valid k
  u = E_beta[k], std = E_beta[(k-u)^2]
  li = (i-u)^2/std + 0.005*ln(std), summed over valid i; total / n_pairs.

Kernel decomposition: work items = 512-query blocks of each direction.
Per item, scores are computed transposed ([keys->partitions, queries->free])
via augmented matmuls so softmax denominators reduce over partitions on the
PE (no running max needed: pass-A scores <= 0; pass-B scores bounded).
Variance uses a two-round pass B (u first, then sum P2*(u-k)^2 elementwise)
to avoid catastrophic cancellation. Items are sorted by size and dealt
8-at-a-time into steps; loop bounds are compile-time per step.
"""
import sys
import numpy as np

sys.path.insert(0, "/opt/trn_rl_repo")

QB = 512          # query block = matmul free dim = one PSUM bank of fp32
KG = 256          # key group (2 chunks of 128 partitions)
NCORES = 8
PENALTY = 0.01
BIG = 1.0e30
STD_FLOOR = 1.0e-35


def _ceil(a, b):
    return -(-a // b)


class _Item:
    __slots__ = ("qi", "ki", "Lq", "Lk", "qb", "ga", "gb", "dummy")

    def __init__(self, qi, ki, Lq, Lk, qb):
        self.qi, self.ki, self.Lq, self.Lk, self.qb = qi, ki, Lq, Lk, qb
        self.ga = _ceil(Lk, KG)
        self.gb = _ceil(Lq, KG)
        self.dummy = False


class _Dummy:
    qi = ki = Lq = Lk = qb = 0
    ga = gb = 0
    dummy = True


def pack(seq, src_len, combinations):
    """Build the step plan and per-core input arrays.

    Per-core inputs (all fp32):
      kA  [34, CA]   pass-A key operand rows [yT; y2; 1] (masked keys y2=BIG)
      vAr [128, CA//128*33]  pass-A values, pre-swizzled so the on-chip
                     [128, 2GA, 33] tile loads with contiguous per-partition
                     rows: vAr[p, g*33+d] = vA[g*128+p, d], vA = [y | 1]
      qA  [34, QB*NS] pass-A query operand rows [2xT; -1; -x2]
      kB  [33, CB]   pass-B key operand rows [2xT; x2] (masked keys x2=BIG)
      kvo [128, 66]  col 2j = global key index of chunk j (j<32), col 2j+1 = 1;
                     col 64 = ones (sum-weights lhsT), col 65 = 0
      qidx/qmask [128, 4*NS] absolute query index / valid mask per B-slot
    """
    seq = np.asarray(seq, np.float32)
    lens = (np.asarray(src_len).astype(np.int64) // 4).astype(np.int64)
    comb = np.asarray(combinations).astype(np.int64)

    items = []
    for a, b in comb:
        for qi, ki in ((a, b), (b, a)):
            Lq, Lk = int(lens[qi]), int(lens[ki])
            if Lq <= 0 or Lk <= 0:
                continue
            for qb in range(_ceil(Lq, QB)):
                items.append(_Item(int(qi), int(ki), Lq, Lk, qb))
    items.sort(key=lambda it: -(it.ga + it.gb))
    NS = max(1, _ceil(len(items), NCORES))
    while len(items) < NS * NCORES:
        items.append(_Dummy())

    GA = [max(max(items[s * NCORES + c].ga for c in range(NCORES)), 1)
          for s in range(NS)]
    GB = [max(max(items[s * NCORES + c].gb for c in range(NCORES)), 1)
          for s in range(NS)]
    CA = sum(GA) * KG
    CB = sum(GB) * KG

    sq2 = np.einsum("btd,btd->bt", seq, seq).astype(np.float32)

    kvo = np.zeros((128, 66), np.float32)
    for j in range(32):
        kvo[:, 2 * j] = (j * 128 + np.arange(128)).astype(np.float32)
        kvo[:, 2 * j + 1] = 1.0
    kvo[:, 64] = 1.0

    cores = []
    for c in range(NCORES):
        kA = np.zeros((34, CA), np.float32)
        vA = np.zeros((CA, 33), np.float32)
        qA = np.zeros((34, QB * NS), np.float32)
        kB = np.zeros((33, CB), np.float32)
        qidx = np.zeros((128, 4 * NS), np.float32)
        qmask = np.zeros((128, 4 * NS), np.float32)
        offa = 0
        offb = 0
        its = []
        for s in range(NS):
            it = items[s * NCORES + c]
            its.append(it)
            na = GA[s] * KG
            nb = GB[s] * KG
            ka = kA[:, offa:offa + na]
            va = vA[offa:offa + na]
            kb = kB[:, offb:offb + nb]
            qa = qA[:, s * QB:(s + 1) * QB]
            if it.dummy:
                ka[33, :] = 1.0
                va[:, 32] = 1.0
            else:
                y = seq[it.ki]
                x = seq[it.qi]
                Lk, Lq = it.Lk, it.Lq
                nk = min(Lk, na)
                ka[0:32, :nk] = y[:nk].T
                ka[32, :nk] = sq2[it.ki, :nk]
                ka[33, :nk] = 1.0
                ka[32, nk:] = BIG
                ka[33, nk:] = 1.0
                va[:nk, 0:32] = y[:nk]
                va[:nk, 32] = 1.0
                q0 = it.qb * QB
                nq = min(Lq - q0, QB)
                qa[0:32, :nq] = 2.0 * x[q0:q0 + nq].T
                qa[32, :nq] = -1.0
                qa[33, :nq] = -sq2[it.qi, q0:q0 + nq]
                nkb = min(Lq, nb)
                kb[0:32, :nkb] = 2.0 * x[:nkb].T
                kb[32, :nkb] = sq2[it.qi, :nkb]
                kb[32, nkb:] = BIG
                for c4 in range(4):
                    ii = q0 + c4 * 128 + np.arange(128)
                    qidx[:, s * 4 + c4] = ii.astype(np.float32)
                    qmask[:, s * 4 + c4] = (ii < Lq).astype(np.float32)
            offa += na
            offb += nb
        vAr = np.ascontiguousarray(
            vA.reshape(CA // 128, 128, 33).transpose(1, 0, 2).reshape(128, -1))
        cores.append(dict(kA=kA, vAr=vAr, qA=qA, kB=kB, kvo=kvo,
                          qidx=qidx, qmask=qmask, items=its))
    plan = dict(NS=NS, GA=GA, GB=GB, CA=CA, CB=CB)
    return plan, cores


def build_program(plan):
    """Build the SPMD Bass program for the given step plan."""
    import concourse.bass as bass
    import concourse.bacc as bacc
    import concourse.mybir as mybir
    import concourse.tile as tile

    F32 = mybir.dt.float32
    AFT = mybir.ActivationFunctionType
    NS, GA, GB = plan["NS"], plan["GA"], plan["GB"]
    CA, CB = plan["CA"], plan["CB"]
    GBmax = max(GB)
    GAmax = max(GA)

    nc = bacc.Bacc("TRN2", target_bir_lowering=False, debug=False,
                   num_devices=NCORES)
    kA_d = nc.dram_tensor("kA", [34, CA], F32, kind="ExternalInput")
    vAr_d = nc.dram_tensor("vAr", [128, (CA // 128) * 33], F32,
                           kind="ExternalInput")
    qA_d = nc.dram_tensor("qA", [34, QB * NS], F32, kind="ExternalInput")
    kB_d = nc.dram_tensor("kB", [33, CB], F32, kind="ExternalInput")
    kvo_d = nc.dram_tensor("kvo", [128, 66], F32, kind="ExternalInput")
    qidx_d = nc.dram_tensor("qidx", [128, 4 * NS], F32, kind="ExternalInput")
    qmask_d = nc.dram_tensor("qmask", [128, 4 * NS], F32, kind="ExternalInput")
    out_d = nc.dram_tensor("out", [1, 1], F32, kind="ExternalOutput")

    with tile.TileContext(nc) as tc:
        with (
            tc.tile_pool(name="keys", bufs=2) as keys_pool,
            tc.tile_pool(name="vals", bufs=2) as vals_pool,
            tc.tile_pool(name="qrys", bufs=2) as qrys_pool,
            tc.tile_pool(name="pa", bufs=2) as pa_pool,
            tc.tile_pool(name="cache", bufs=1) as cache_pool,
            tc.tile_pool(name="epi", bufs=1) as epi_pool,
            tc.tile_pool(name="b2", bufs=2) as b2_pool,
            tc.tile_pool(name="fin", bufs=1) as fin_pool,
            tc.tile_pool(name="sc_ps", bufs=2, space="PSUM") as sc_psum,
            tc.tile_pool(name="na_ps", bufs=1, space="PSUM") as na_psum,
            tc.tile_pool(name="t_ps", bufs=1, space="PSUM") as t_psum,
            tc.tile_pool(name="sd_ps", bufs=1, space="PSUM") as sd_psum,
        ):
            kvo = fin_pool.tile([128, 66], F32)
            nc.sync.dma_start(kvo[:], kvo_d[:])
            qidx = fin_pool.tile([128, 4 * NS], F32)
            nc.sync.dma_start(qidx[:], qidx_d[:])
            qmask = fin_pool.tile([128, 4 * NS], F32)
            nc.sync.dma_start(qmask[:], qmask_d[:])
            stats_u = fin_pool.tile([128, 4 * NS], F32)
            stats_s = fin_pool.tile([128, 4 * NS], F32)

            offa = 0
            offb = 0
            for s in range(NS):
                ga, gb = GA[s], GB[s]
                na, nb = ga * KG, gb * KG
                # ---- load this step's operands
                kA_t = keys_pool.tile([34, GAmax * KG], F32, tag="kA")
                nc.sync.dma_start(kA_t[:, :na], kA_d[:, offa:offa + na])
                vA_t = vals_pool.tile([128, GAmax * 2 * 33], F32, tag="vA")
                nc.sync.dma_start(
                    vA_t[:, :ga * 66],
                    vAr_d[:, (offa // 128) * 33:((offa + na) // 128) * 33])
                qA_t = qrys_pool.tile([34, QB], F32, tag="qA")
                nc.sync.dma_start(qA_t[:], qA_d[:, s * QB:(s + 1) * QB])
                kB_t = keys_pool.tile([33, GBmax * KG], F32, tag="kB")
                nc.sync.dma_start(kB_t[:, :nb], kB_d[:, offb:offb + nb])

                # ---- pass A: numA[0:32] = snn.T * Z, numA[32] = Z
                numA = na_psum.tile([33, QB], F32)
                for g in range(ga):
                    sc = sc_psum.tile([128, 2 * QB], F32, tag="sc")
                    P = pa_pool.tile([128, 2 * QB], F32, tag="pa")
                    for h in range(2):
                        ch = 2 * g + h
                        nc.tensor.matmul(
                            sc[:, h * QB:(h + 1) * QB],
                            kA_t[:, ch * 128:(ch + 1) * 128], qA_t[:],
                            start=True, stop=True)
                    nc.scalar.activation(P[:], sc[:], AFT.Exp)
                    for h in range(2):
                        ch = 2 * g + h
                        nc.tensor.matmul(
                            numA[:],
                            vA_t[:, ch * 33:(ch + 1) * 33],
                            P[:, h * QB:(h + 1) * QB],
                            start=(g == 0 and h == 0),
                            stop=(g == ga - 1 and h == 1))

                # ---- epilogue A: R2 = [snn.T; -1]
                nsb = epi_pool.tile([33, QB], F32, tag="nsb")
                nc.vector.tensor_copy(nsb[:], numA[:])
                zrow = epi_pool.tile([1, QB], F32, tag="zrow")
                nc.sync.dma_start(zrow[:], nsb[32:33, :])
                rz0 = epi_pool.tile([1, QB], F32, tag="rz0")
                nc.vector.reciprocal(rz0[:], zrow[:])
                rb = epi_pool.tile([32, QB], F32, tag="rb")
                nc.gpsimd.partition_broadcast(rb[:], rz0[:])
                R2 = epi_pool.tile([33, QB], F32, tag="R2")
                nc.gpsimd.memset(R2[32:33, :], -1.0)
                nc.vector.tensor_mul(R2[0:32, :], nsb[0:32, :], rb[:])

                # ---- pass B1: P2 cached; T = [r0; Z2]
                cache = cache_pool.tile([128, GBmax * 2 * QB], F32, tag="p2c")
                T = t_psum.tile([2, QB], F32, tag="T")
                for g in range(gb):
                    sc = sc_psum.tile([128, 2 * QB], F32, tag="sc")
                    for h in range(2):
                        ch = 2 * g + h
                        nc.tensor.matmul(
                            sc[:, h * QB:(h + 1) * QB],
                            kB_t[:, ch * 128:(ch + 1) * 128], R2[:],
                            start=True, stop=True)
                    nc.scalar.activation(
                        cache[:, g * 2 * QB:(g + 1) * 2 * QB], sc[:], AFT.Exp)
                    for h in range(2):
                        ch = 2 * g + h
                        nc.tensor.matmul(
                            T[:],
                            kvo[:, 2 * ch:2 * ch + 2],
                            cache[:, (2 * g + h) * QB:(2 * g + h + 1) * QB],
                            start=(g == 0 and h == 0),
                            stop=(g == gb - 1 and h == 1))

                # ---- mid: u = r0 / Z2, broadcast
                tt = epi_pool.tile([2, QB], F32, tag="tt")
                nc.vector.tensor_copy(tt[:], T[:])
                z2row = epi_pool.tile([1, QB], F32, tag="z2row")
                nc.sync.dma_start(z2row[:], tt[1:2, :])
                rz2 = epi_pool.tile([1, QB], F32, tag="rz2")
                nc.vector.reciprocal(rz2[:], z2row[:])
                u0 = epi_pool.tile([1, QB], F32, tag="u0")
                nc.vector.tensor_mul(u0[:], tt[0:1, :], rz2[:])
                ub = epi_pool.tile([128, QB], F32, tag="ub")
                nc.gpsimd.partition_broadcast(ub[:], u0[:])

                # ---- pass B2: stdsum = sum_k P2 * (u-k)^2
                stdsum = sd_psum.tile([1, QB], F32, tag="sd")
                for g in range(gb):
                    for h in range(2):
                        ch = 2 * g + h
                        d = b2_pool.tile([128, QB], F32, tag="d")
                        nc.vector.tensor_scalar_sub(
                            d[:], ub[:], kvo[:, 2 * ch:2 * ch + 1])
                        sq = b2_pool.tile([128, QB], F32, tag="sq")
                        nc.vector.tensor_mul(sq[:], d[:], d[:])
                        w = b2_pool.tile([128, QB], F32, tag="w")
                        nc.gpsimd.tensor_mul(
                            w[:], sq[:],
                            cache[:, (2 * g + h) * QB:(2 * g + h + 1) * QB])
                        nc.tensor.matmul(
                            stdsum[:], kvo[:, 64:65], w[:],
                            start=(g == 0 and h == 0),
                            stop=(g == gb - 1 and h == 1))

                # ---- epilogue B: write u, std into stats via transpose-DMA
                sstd = epi_pool.tile([1, QB], F32, tag="sstd")
                nc.vector.tensor_mul(sstd[:], stdsum[:], rz2[:])
                for c4 in range(4):
                    nc.sync.dma_start(
                        stats_u[:, s * 4 + c4:s * 4 + c4 + 1],
                        u0[0:1, c4 * 128:(c4 + 1) * 128])
                    nc.sync.dma_start(
                        stats_s[:, s * 4 + c4:s * 4 + c4 + 1],
                        sstd[0:1, c4 * 128:(c4 + 1) * 128])
                offa += na
                offb += nb

            # ---- final: li = (i-u)^2/std + 0.005*ln(std), masked sum
            W = 4 * NS
            stdc = fin_pool.tile([128, W], F32)
            nc.vector.tensor_scalar_max(stdc[:], stats_s[:], STD_FLOOR)
            rstd = fin_pool.tile([128, W], F32)
            nc.vector.reciprocal(rstd[:], stdc[:])
            delta = fin_pool.tile([128, W], F32)
            nc.vector.tensor_sub(delta[:], qidx[:], stats_u[:])
            d2 = fin_pool.tile([128, W], F32)
            nc.vector.tensor_mul(d2[:], delta[:], delta[:])
            t1 = fin_pool.tile([128, W], F32)
            nc.vector.tensor_mul(t1[:], d2[:], rstd[:])
            lg = fin_pool.tile([128, W], F32)
            nc.scalar.activation(lg[:], stdc[:], AFT.Ln)
            lgs = fin_pool.tile([128, W], F32)
            nc.vector.tensor_scalar_mul(lgs[:], lg[:], 0.5 * PENALTY)
            li = fin_pool.tile([128, W], F32)
            nc.vector.tensor_add(li[:], t1[:], lgs[:])
            lim = fin_pool.tile([128, W], F32)
            nc.vector.tensor_mul(lim[:], li[:], qmask[:])
            rowsum = fin_pool.tile([128, 1], F32)
            nc.vector.reduce_sum(rowsum[:], lim[:],
                                 axis=mybir.AxisListType.X)
            tot = t_psum.tile([1, 1], F32, tag="tot")
            nc.tensor.matmul(tot[:], rowsum[:], kvo[0:128, 64:65],
                             start=True, stop=True)
            osb = fin_pool.tile([1, 1], F32)
            nc.vector.tensor_copy(osb[:], tot[:])
            nc.sync.dma_start(out_d[:], osb[:])

    nc.compile()
    return nc


def kernel(seq, src_len, combinations):
    from concourse.bass_utils import run_bass_kernel_spmd

    plan, cores = pack(seq, src_len, combinations)
    nc = build_program(plan)
    in_maps = [
        {k: ci[k] for k in
         ("kA", "vAr", "qA", "kB", "kvo", "qidx", "qmask")}
        for ci in cores
    ]
    res = run_bass_kernel_spmd(nc, in_maps, list(range(NCORES)))
    tot = np.float32(0.0)
    for c in range(NCORES):
        tot += np.float32(res.results[c]["out"][0, 0])
    n_pairs = np.asarray(combinations).shape[0]
    return np.float32(tot / np.float32(n_pairs))

